# revision 1
# baseline (speedup 1.0000x reference)
"""Trainium2 Bass kernel for nn_CruxMiniCircuit (gnn_message_passing).

Reference semantics: B independent rows; each row is a circuit of N nodes
(literal nodes hold a fixed one-hot distribution over 10 ints, op nodes
combine left/right child distributions through a per-op bilinear table
followed by softmax).  The reference runs 10 synchronous passes over all
nodes and returns only the root (node 0) logits per row.

Key observation: the output depends only on node 0's dependency cone
unrolled 10 passes deep.  Literal children are compile-time constants
(one-hot vectors) and op nodes at pass 0 are zero, so the per-row
worklists are tiny (mean ~5 updates/row for the benchmark distribution).
The host precomputes integer worklists / gather indices; the device does
all floating-point math.

Device pipeline per pass: ap_gather pulls child value vectors out of
per-row-group SBUF value buffers (rows are binned into 8 groups so all 8
GPSIMD Q7 cores gather concurrently); small DMAs concatenate the
group-blocked gather output into contraction layout; TensorE builds the
replicated operands and the bilinear contraction; softmax runs as
exp (ScalarE) + ones-matmul reduction (TensorE) + reciprocal (VectorE);
all three per-op softmax results are stored so op selection folds into
the next pass's gather indices.  Pass-1 inputs are constants and are
shipped from the host directly, skipping one gather.

Sharding: pure data parallel over the batch dim (B=2048 -> 256 rows on
each of the 8 NeuronCores), per the sharding hint.  No collectives are
needed for the forward pass.
"""

import sys
from contextlib import ExitStack

import numpy as np

sys.path.insert(0, "/opt/trn_rl_repo")

import concourse.bass as bass
import concourse.tile as tile
from concourse import bacc, mybir
from concourse.bass_utils import run_bass_kernel_spmd

B, N = 2048, 1023
NI, NO, NP = 10, 3, 10  # n_ints, n_ops, n_passes
NCORES = 8
RPC = B // NCORES  # rows per core
NG = 8  # row groups per core (one per Q7 core / 16-partition block)
ZSLOT = NI  # value-buffer slot holding the zero vector
NCONST = NI + 1  # slots 0..9 = one-hot e_k, slot 10 = zeros
CHUNK = 448  # free-dim chunk for the compute pipeline (PSUM/matmul limits)

TRACE = False  # set True (e.g. from test.py) to profile the HW run
LAST_RESULTS = None  # BassKernelResults of the last run (exec_time_ns etc.)


def _plan(cats, ops, lits, left, right, mask):
    """Integer-only preprocessing: worklists, group binning, gather indices."""
    left = np.clip(left.astype(np.int64), 0, N - 1)
    right = np.clip(right.astype(np.int64), 0, N - 1)
    opsc = np.clip(ops.astype(np.int64), 0, NO - 1)
    litsc = np.clip(lits.astype(np.int64), 0, NI - 1)
    m = mask.astype(bool)
    is_lit = (cats == 0) & m
    is_opa = (cats == 1) & m
    const_slot = np.where(is_lit, litsc, ZSLOT)

    # Worklists W[p]: (row, node) updates needed at pass p.
    Wr = [None] * (NP + 1)
    Wn = [None] * (NP + 1)
    r10 = np.nonzero(cats[:, 0] == 1)[0].astype(np.int64)
    Wr[NP], Wn[NP] = r10, np.zeros(len(r10), np.int64)
    need = np.zeros((B, N), bool)
    for p in range(NP, 1, -1):
        r, n = Wr[p], Wn[p]
        cr = np.concatenate([r, r])
        cn = np.concatenate([left[r, n], right[r, n]])
        keep = is_opa[cr, cn]
        need[:] = False
        need[cr[keep], cn[keep]] = True
        rr, nn = np.nonzero(need)
        Wr[p - 1], Wn[p - 1] = rr.astype(np.int64), nn.astype(np.int64)

    # Bin rows into NG groups per core, balancing total updates per group.
    weight = np.zeros(B, np.int64)
    for p in range(1, NP + 1):
        np.add.at(weight, Wr[p], 1)
    G = np.zeros(B, np.int64)
    for c in range(NCORES):
        rows = np.arange(c * RPC, (c + 1) * RPC)
        order = rows[np.argsort(-weight[rows], kind="stable")]
        load = np.zeros(NG, np.int64)
        for rr_ in order:
            g = int(load.argmin())
            G[rr_] = g
            load[g] += weight[rr_]

    # Per-pass group-local ids and padded per-group size Q_p.
    Qp = np.zeros(NP + 1, np.int64)
    gid = [None] * (NP + 1)
    for p in range(1, NP + 1):
        r = Wr[p]
        core = r // RPC
        grp = G[r]
        key = core * NG + grp
        order = np.argsort(key, kind="stable")
        ks = key[order]
        u = np.arange(len(ks), dtype=np.int64)
        if len(ks):
            first = np.r_[True, ks[1:] != ks[:-1]]
            seg_idx = np.nonzero(first)[0]
            u = u - seg_idx[np.cumsum(first) - 1]
        ul = np.empty(len(ks), np.int64)
        ul[order] = u
        cnt = np.bincount(key, minlength=NCORES * NG) if len(r) else np.zeros(NCORES * NG, np.int64)
        mx = int(cnt.max()) if len(r) else 0
        Qp[p] = max(8, -(-mx // 8) * 8)  # multiple of 8 -> num_idxs % 16 == 0
        gid[p] = (core, grp, ul)

    # Buffer slot bases (group-local numbering); passes 1..NP-1 store 3 slots/update.
    base = np.zeros(NP + 1, np.int64)
    base[1] = NCONST
    for p in range(2, NP + 1):
        base[p] = base[p - 1] + 3 * Qp[p - 1]
    S = int(base[NP - 1] + 3 * Qp[NP - 1])
    assert S <= 32000, f"value buffer too large for int16 gather indices: {S}"

    idx_wrapped = []
    Ftot = 0
    slot_prev = np.full((B, N), -1, np.int64)
    lr1 = None
    for p in range(1, NP + 1):
        r, n = Wr[p], Wn[p]
        core, grp, ul = gid[p]
        lch, rch = left[r, n], right[r, n]
        if p == 1:
            lidx = const_slot[r, lch]
            ridx = const_slot[r, rch]
        else:
            lidx = np.where(is_opa[r, lch],
                            base[p - 1] + 3 * slot_prev[r, lch] + opsc[r, lch],
                            const_slot[r, lch])
            ridx = np.where(is_opa[r, rch],
                            base[p - 1] + 3 * slot_prev[r, rch] + opsc[r, rch],
                            const_slot[r, rch])
        Q = int(Qp[p])
        arr = np.full((NCORES, NG, 2 * Q), ZSLOT, np.int64)
        arr[core, grp, ul] = lidx
        arr[core, grp, Q + ul] = ridx
        if p == 1:
            # pass-1 inputs are constants; ship lr1 from host (skip the gather).
            # lr10 layout: (10, 2*NG*Q): l half col g*Q+u ; r half col NG*Q+g*Q+u
            eyeext = np.concatenate([np.eye(NI, dtype=np.float32),
                                     np.zeros((NI, 1), np.float32)], axis=1)
            cols = arr.reshape(NCORES, NG, 2, Q).transpose(0, 2, 1, 3).reshape(NCORES, 2 * NG * Q)
            lr1 = np.ascontiguousarray(eyeext[:, cols].transpose(1, 0, 2))  # (NCORES, 10, 2*NG*Q)
        else:
            F = -(-2 * Q // 16)
            F += F & 1  # 4-byte-aligned idx slices (ucode reads dwords)
            tmp = np.full((NCORES, NG, F * 16), ZSLOT, np.int64)
            tmp[:, :, : 2 * Q] = arr
            w = tmp.reshape(NCORES, NG, F, 16).transpose(0, 1, 3, 2).reshape(NCORES, NG * 16, F)
            idx_wrapped.append(w.astype(np.int16))
            Ftot += F
        if p < NP:
            slot_prev = np.full((B, N), -1, np.int64)
            slot_prev[r, n] = ul

    idx_full = np.concatenate(idx_wrapped, axis=2)  # (NCORES, 128, Ftot)

    return dict(
        Qp=Qp, base=base, S=S, idx=idx_full, Ftot=Ftot, lr1=lr1,
        r10=r10, gid10=gid[NP],
        opsc=opsc, litsc=litsc, is_lit=is_lit, m=m, G=G,
    )


_CUR_BASE = None


def _build_nc(S, Qp, Ftot):
    f32 = mybir.dt.float32
    Q10 = int(Qp[NP])
    PT10 = NG * Q10
    nc = bacc.Bacc(None)
    consts = nc.dram_tensor("consts", [NI, NCONST], f32, kind="ExternalInput")
    wmat = nc.dram_tensor("wmat", [100, 74], f32, kind="ExternalInput")
    repl = nc.dram_tensor("repl", [NI, 100], f32, kind="ExternalInput")
    reprm = nc.dram_tensor("reprm", [NI, 100], f32, kind="ExternalInput")
    oblk = nc.dram_tensor("oblk", [74, NO], f32, kind="ExternalInput")
    oblk2 = nc.dram_tensor("oblk2", [NO, 74], f32, kind="ExternalInput")
    idx_in = nc.dram_tensor("idx", [128, Ftot], mybir.dt.int16, kind="ExternalInput")
    PT1 = NG * int(Qp[1])
    lr1_in = nc.dram_tensor("lr1", [NI, 2 * PT1], f32, kind="ExternalInput")
    outz = nc.dram_tensor("outz", [74, PT10], f32, kind="ExternalOutput")

    with ExitStack() as ctx:
        tc = ctx.enter_context(tile.TileContext(nc))
        singles = ctx.enter_context(tc.tile_pool(name="singles", bufs=1))
        work = ctx.enter_context(tc.tile_pool(name="work", bufs=2))
        psum = ctx.enter_context(tc.tile_pool(name="psum", bufs=1, space="PSUM"))
        lrpool = ctx.enter_context(tc.tile_pool(name="lrpool", bufs=1))

        buf = singles.tile([128, S], f32)
        nc.vector.memset(buf[:, :], 0.0)
        for g in range(NG):
            nc.sync.dma_start(out=buf[16 * g : 16 * g + NI, 0:NCONST], in_=consts[:, :])
        w_sb = singles.tile([100, 74], f32)
        nc.sync.dma_start(out=w_sb[:, :], in_=wmat[:, :])
        repl_sb = singles.tile([NI, 100], f32)
        nc.sync.dma_start(out=repl_sb[:, :], in_=repl[:, :])
        reprm_sb = singles.tile([NI, 100], f32)
        nc.sync.dma_start(out=reprm_sb[:, :], in_=reprm[:, :])
        oblk_sb = singles.tile([74, NO], f32)
        nc.sync.dma_start(out=oblk_sb[:, :], in_=oblk[:, :])
        oblk2_sb = singles.tile([NO, 74], f32)
        nc.sync.dma_start(out=oblk2_sb[:, :], in_=oblk2[:, :])
        idx_sb = singles.tile([128, Ftot], mybir.dt.int16)
        nc.sync.dma_start(out=idx_sb[:, :], in_=idx_in[:, :])

        foff = 0
        for p in range(1, NP + 1):
            Q = int(Qp[p])
            PT = NG * Q
            lr10 = lrpool.tile([NI, 2 * PT], f32, tag=f"lr10_{p}")
            if p == 1:
                nc.sync.dma_start(out=lr10[:, :], in_=lr1_in[:, :])
            else:
                F = -(-2 * Q // 16)
                F += F & 1
                lrg = lrpool.tile([128, 2 * Q], f32, tag=f"lrg{p}")
                nc.gpsimd.ap_gather(
                    out_ap=lrg[:, :],
                    in_ap=buf[:, :],
                    idxs_ap=idx_sb[:, foff : foff + F],
                    channels=128,
                    num_elems=S,
                    d=1,
                    num_idxs=2 * Q,
                )
                foff += F
                # concat groups: lr10[i, h*PT + g*Q + u] = lrg[16g+i, h*Q + u]
                for g in range(NG):
                    src = lrg[16 * g : 16 * g + NI, :].rearrange("i (h u) -> i h u", h=2)
                    dst = lr10[:, :].rearrange("i (h gg u) -> i h gg u", h=2, gg=NG)[:, :, g, :]
                    nc.sync.dma_start(out=dst, in_=src)
            for c0 in range(0, PT, CHUNK):
                cw = min(CHUNK, PT - c0)
                ps_l = psum.tile([100, cw], f32, tag="ps_l")
                nc.tensor.matmul(ps_l[:, :], repl_sb[:, :], lr10[:, c0 : c0 + cw],
                                 start=True, stop=True)
                ps_r = psum.tile([100, cw], f32, tag="ps_r")
                nc.tensor.matmul(ps_r[:, :], reprm_sb[:, :], lr10[:, PT + c0 : PT + c0 + cw],
                                 start=True, stop=True)
                lrep_sb = work.tile([100, cw], f32, tag="lrep_sb")
                nc.vector.tensor_copy(lrep_sb[:, :], ps_l[:, :])
                outer = work.tile([100, cw], f32, tag="outer")
                nc.vector.tensor_mul(outer[:, :], lrep_sb[:, :], ps_r[:, :])
                ps_z = psum.tile([74, cw], f32, tag="ps_z")
                nc.tensor.matmul(ps_z[:, :], w_sb[:, :], outer[:, :], start=True, stop=True)
                if p == NP:
                    zsb = work.tile([74, cw], f32, tag="zsb")
                    nc.scalar.copy(zsb[:, :], ps_z[:, :])
                    nc.sync.dma_start(out=outz[:, c0 : c0 + cw], in_=zsb[:, :])
                    continue
                e = work.tile([74, cw], f32, tag="e")
                nc.scalar.activation(e[:, :], ps_z[:, :], mybir.ActivationFunctionType.Exp)
                ps_z3 = psum.tile([NO, cw], f32, tag="ps_z3")
                nc.tensor.matmul(ps_z3[:, :], oblk_sb[:, :], e[:, :], start=True, stop=True)
                rz = work.tile([NO, cw], f32, tag="rz")
                nc.vector.reciprocal(rz[:, :], ps_z3[:, :])
                ps_rz = psum.tile([74, cw], f32, tag="ps_rz")
                nc.tensor.matmul(ps_rz[:, :], oblk2_sb[:, :], rz[:, :], start=True, stop=True)
                st = work.tile([NI, 3 * cw], f32, tag="st")
                for o in range(NO):
                    nc.vector.tensor_mul(
                        st[:, o : 3 * cw : 3],
                        e[o * 32 : o * 32 + NI, :],
                        ps_rz[o * 32 : o * 32 + NI, :],
                    )
                # scatter back: buf[16g+k, b0+3u+o] = st[k, 3*(g*Q+u)+o]
                b0 = int(_CUR_BASE[p])
                for g in range(NG):
                    nc.sync.dma_start(
                        out=buf[16 * g : 16 * g + NI, b0 : b0 + 3 * Q],
                        in_=st[:, 3 * g * Q : 3 * (g + 1) * Q],
                    )
    nc.finalize()
    return nc


def kernel(op_table, cats, ops, lits, left, right, mask):
    global _CUR_BASE, LAST_RESULTS
    op_table = np.asarray(op_table, np.float32)
    plan = _plan(np.asarray(cats), np.asarray(ops), np.asarray(lits),
                 np.asarray(left), np.asarray(right), np.asarray(mask))
    Qp, base, S, Ftot = plan["Qp"], plan["base"], plan["S"], plan["Ftot"]
    _CUR_BASE = base
    assert NG * int(max(Qp[1:])) <= CHUNK, f"chunking not supported: {Qp}"

    nc = _build_nc(S, Qp, Ftot)

    consts = np.concatenate([np.eye(NI, dtype=np.float32),
                             np.zeros((NI, 1), np.float32)], axis=1)
    wmat = np.zeros((100, 74), np.float32)
    w30 = op_table.transpose(1, 2, 0, 3).reshape(100, 30)
    oblk = np.zeros((74, NO), np.float32)
    oblk2 = np.zeros((NO, 74), np.float32)
    for o in range(NO):
        wmat[:, o * 32 : o * 32 + NI] = w30[:, o * NI : (o + 1) * NI]
        oblk[o * 32 : o * 32 + NI, o] = 1.0
        oblk2[o, o * 32 : o * 32 + NI] = 1.0
    repl = np.kron(np.eye(NI), np.ones((1, NI))).astype(np.float32)
    reprm = np.tile(np.eye(NI), (1, NI)).astype(np.float32)

    in_maps = []
    for c in range(NCORES):
        in_maps.append({
            "consts": consts, "wmat": wmat, "repl": repl, "reprm": reprm,
            "oblk": oblk, "oblk2": oblk2,
            "idx": np.ascontiguousarray(plan["idx"][c]),
            "lr1": np.ascontiguousarray(plan["lr1"][c]),
        })

    res = run_bass_kernel_spmd(nc, in_maps, list(range(NCORES)), trace=TRACE)
    LAST_RESULTS = res

    # Assemble the full (B, NI) output on the host (index selection only).
    out = np.zeros((B, NI), np.float32)
    litsc, is_lit = plan["litsc"], plan["is_lit"]
    lit_rows = np.nonzero(cats[:, 0] == 0)[0]
    lr_active = is_lit[lit_rows, 0]
    oh = 10.0 * np.eye(NI, dtype=np.float32)[litsc[lit_rows, 0]]
    out[lit_rows] = np.where(lr_active[:, None], oh, 0.0)

    r10, opsc = plan["r10"], plan["opsc"]
    core10, grp10, ul10 = plan["gid10"]
    Q10 = int(Qp[NP])
    cols = grp10 * Q10 + ul10
    for c in range(NCORES):
        z = np.asarray(res.results[c]["outz"])  # (74, PT10)
        selmask = core10 == c
        rows = r10[selmask]
        cc = cols[selmask]
        o = opsc[rows, 0]
        zc = z[:, cc]
        sel = np.stack([zc[i * 32 : i * 32 + NI, :] for i in range(NO)])
        out[rows] = sel[o, :, np.arange(len(rows))]
    return out



# revision 6
# speedup vs baseline: 1.4222x; 1.4222x over previous
"""Trainium2 Bass kernel for nn_CruxMiniCircuit (gnn_message_passing).

Reference semantics: B independent rows; each row is a circuit of N nodes
(literal nodes hold a fixed one-hot distribution over 10 ints, op nodes
combine left/right child distributions through a per-op bilinear table
followed by softmax).  The reference runs 10 synchronous passes over all
nodes and returns only the root (node 0) logits per row.

Only node 0's dependency cone matters, so the host precomputes per-pass
integer worklists / gather indices; the device does all float math.

v2 design (latency-oriented; the v1 baseline spent most of the critical
path issuing 16 SBUF->SBUF DMAs per pass from the sync engine):

- The per-core value buffer buf[128, S] is REPLICATED across the eight
  16-partition GPSIMD blocks: the scatter at the end of each pass is a
  TensorE broadcast matmul (stationary E_o[74->128]) into PSUM followed
  by one ScalarE copy per op slice.  No DMAs inside the pass loop.
- The per-pass child gather is ONE ap_gather over all 128 channels with
  num_idxs = 2*PT: block g's channels carry real slot indices for the
  columns block g owns and ZSLOT (a zero vector) elsewhere.  A stationary
  Wl/Wr[128, 100] with the replicate pattern repeated per block then sums
  blockwise-disjoint contributions, yielding the left/right operand
  replication in one matmul each - the group-concat DMAs of v1 vanish.
- softmax: exp (ScalarE) -> block-sum matmul M74[74->74] -> reciprocal +
  multiply (VectorE) -> broadcast-scatter matmuls.
- Slot numbering is global per pass (no per-group padding): slot of
  update u with op o at pass p is base[p] + o*PT_p + u.

Sharding: pure data parallel over the batch dim (B=2048 -> 256 rows on
each of the 8 NeuronCores).  No collectives are needed.
"""

import sys
from contextlib import ExitStack

import numpy as np

sys.path.insert(0, "/opt/trn_rl_repo")

import concourse.bass as bass
import concourse.tile as tile
from concourse import bacc, mybir
from concourse.bass_utils import run_bass_kernel_spmd

B, N = 2048, 1023
NI, NO, NP = 10, 3, 10  # n_ints, n_ops, n_passes
NCORES = 8
RPC = B // NCORES  # rows per core
ZSLOT = NI  # value-buffer slot holding the zero vector
NCONST = NI + 1  # slots 0..9 = one-hot e_k, slot 10 = zeros

TRACE = False  # set True (e.g. from test.py) to profile the HW run
LAST_RESULTS = None  # BassKernelResults of the last run (exec_time_ns etc.)


def _plan(cats, ops, lits, left, right, mask):
    """Integer-only preprocessing: worklists, column assignment, gather idxs."""
    left = np.clip(left.astype(np.int64), 0, N - 1)
    right = np.clip(right.astype(np.int64), 0, N - 1)
    opsc = np.clip(ops.astype(np.int64), 0, NO - 1)
    litsc = np.clip(lits.astype(np.int64), 0, NI - 1)
    m = mask.astype(bool)
    is_lit = (cats == 0) & m
    is_opa = (cats == 1) & m
    const_slot = np.where(is_lit, litsc, ZSLOT)

    # Worklists W[p]: (row, node) updates needed at pass p.
    Wr = [None] * (NP + 1)
    Wn = [None] * (NP + 1)
    r10 = np.nonzero(cats[:, 0] == 1)[0].astype(np.int64)
    Wr[NP], Wn[NP] = r10, np.zeros(len(r10), np.int64)
    need = np.zeros((B, N), bool)
    for p in range(NP, 1, -1):
        r, n = Wr[p], Wn[p]
        cr = np.concatenate([r, r])
        cn = np.concatenate([left[r, n], right[r, n]])
        keep = is_opa[cr, cn]
        need[:] = False
        need[cr[keep], cn[keep]] = True
        rr, nn = np.nonzero(need)
        Wr[p - 1], Wn[p - 1] = rr.astype(np.int64), nn.astype(np.int64)

    # Per-core column ids (order of appearance) and per-pass padded width PT.
    PT = np.zeros(NP + 1, np.int64)
    ucol = [None] * (NP + 1)
    for p in range(1, NP + 1):
        r = Wr[p]
        core = r // RPC
        order = np.argsort(core, kind="stable")
        u = np.arange(len(r), dtype=np.int64)
        if len(r):
            cs = core[order]
            first = np.r_[True, cs[1:] != cs[:-1]]
            seg = np.nonzero(first)[0]
            uu = u - seg[np.cumsum(first) - 1]
            ul = np.empty(len(r), np.int64)
            ul[order] = uu
        else:
            ul = u
        cnt = np.bincount(core, minlength=NCORES) if len(r) else np.zeros(NCORES, np.int64)
        mx = int(cnt.max()) if len(r) else 0
        PT[p] = max(16, -(-mx // 16) * 16)  # mult of 16 -> idx words fill F exactly
        ucol[p] = ul

    # Slot bases: pass p stores 3*PT[p] slots (p = 1..NP-1).
    base = np.zeros(NP + 1, np.int64)
    base[1] = NCONST
    for p in range(2, NP + 1):
        base[p] = base[p - 1] + 3 * PT[p - 1]
    S = int(base[NP - 1] + 3 * PT[NP - 1])
    assert S <= 32000, f"value buffer too large for gather: {S}"

    idx_parts = []
    Fs = []
    slot_prev = np.full((B, N), -1, np.int64)
    lr1 = None
    for p in range(1, NP + 1):
        r, n, ul = Wr[p], Wn[p], ucol[p]
        core = r // RPC
        P = int(PT[p])
        Q8 = P // 8  # columns per 16-partition block (may not be mult of 8; fine)
        g = np.minimum(ul // max(Q8, 1), 7)
        lch, rch = left[r, n], right[r, n]
        if p == 1:
            lidx = const_slot[r, lch]
            ridx = const_slot[r, rch]
        else:
            lidx = np.where(is_opa[r, lch],
                            base[p - 1] + opsc[r, lch] * PT[p - 1] + slot_prev[r, lch],
                            const_slot[r, lch])
            ridx = np.where(is_opa[r, rch],
                            base[p - 1] + opsc[r, rch] * PT[p - 1] + slot_prev[r, rch],
                            const_slot[r, rch])
        if p == 1:
            # pass-1 inputs are compile-time one-hots; built host-side, shipped
            # replicated into the block layout: lr1[c, 16g+i, u] for block g.
            eyeext = np.concatenate([np.eye(NI, dtype=np.float32),
                                     np.zeros((NI, 1), np.float32)], axis=1)
            arr = np.full((NCORES, 2 * P), ZSLOT, np.int64)
            arr[core, ul] = lidx
            arr[core, P + ul] = ridx
            gcol = np.minimum(np.concatenate([np.arange(P), np.arange(P)]) // max(Q8, 1), 7)
            oh = eyeext[:, arr]  # (10, NCORES, 2P)
            big = np.zeros((NCORES, 8, 16, 2 * P), np.float32)
            for gg in range(8):
                sel = gcol == gg
                big[:, gg, :NI, sel] = oh[:, :, sel].transpose(2, 1, 0)
            lr1 = big.reshape(NCORES, 128, 2 * P)
        else:
            # Gather idx array: block g carries real slots for its own column
            # span, ZSLOT elsewhere.  Wrapped: idx j -> [16g + j%16, j//16].
            F = 2 * P // 16
            arr = np.full((NCORES, 8, 2 * P), ZSLOT, np.int64)
            arr[core, g, ul] = lidx
            arr[core, g, P + ul] = ridx
            w = arr.reshape(NCORES, 8, F, 16).transpose(0, 1, 3, 2).reshape(NCORES, 128, F)
            idx_parts.append(w.astype(np.int16))
            Fs.append(F)
        if p < NP:
            slot_prev = np.full((B, N), -1, np.int64)
            slot_prev[r, n] = ul

    idx_full = np.concatenate(idx_parts, axis=2)  # (NCORES, 128, sum F)

    return dict(
        PT=PT, base=base, S=S, idx=idx_full, Ftot=int(sum(Fs)), lr1=lr1,
        r10=r10, core10=Wr[NP] // RPC, ucol10=ucol[NP],
        opsc=opsc, litsc=litsc, is_lit=is_lit,
    )


def _build_nc(S, PT, Ftot):
    f32 = mybir.dt.float32
    nc = bacc.Bacc(None)
    PTmax = int(max(PT[1:]))
    P1 = int(PT[1])
    P10 = int(PT[NP])

    consts = nc.dram_tensor("consts", [128, NCONST], f32, kind="ExternalInput")
    wl = nc.dram_tensor("wl", [128, 100], f32, kind="ExternalInput")
    wr = nc.dram_tensor("wr", [128, 100], f32, kind="ExternalInput")
    wmat = nc.dram_tensor("wmat", [100, 74], f32, kind="ExternalInput")
    m74 = nc.dram_tensor("m74", [74, 74], f32, kind="ExternalInput")
    eblk = nc.dram_tensor("eblk", [74, 3 * 128], f32, kind="ExternalInput")
    idx_in = nc.dram_tensor("idx", [128, Ftot], mybir.dt.int16, kind="ExternalInput")
    lr1_in = nc.dram_tensor("lr1", [128, 2 * P1], f32, kind="ExternalInput")
    outz = nc.dram_tensor("outz", [74, P10], f32, kind="ExternalOutput")

    with ExitStack() as ctx:
        tc = ctx.enter_context(tile.TileContext(nc))
        singles = ctx.enter_context(tc.tile_pool(name="singles", bufs=1))
        work = ctx.enter_context(tc.tile_pool(name="work", bufs=1))
        psum = ctx.enter_context(tc.tile_pool(name="psum", bufs=1, space="PSUM"))

        buf = singles.tile([128, S], f32)
        # No full memset: the gather only ever touches the consts region and
        # slots written by the previous pass's scatter, so only consts need
        # initialization (DMA'd below).
        wl_sb = singles.tile([128, 100], f32)
        nc.sync.dma_start(out=wl_sb[:, :], in_=wl[:, :])
        wr_sb = singles.tile([128, 100], f32)
        nc.sync.dma_start(out=wr_sb[:, :], in_=wr[:, :])
        lr1_sb = singles.tile([128, 2 * P1], f32)
        nc.sync.dma_start(out=lr1_sb[:, :], in_=lr1_in[:, :])
        wmat_sb = singles.tile([100, 74], f32)
        nc.sync.dma_start(out=wmat_sb[:, :], in_=wmat[:, :])
        m74_sb = singles.tile([74, 74], f32)
        nc.sync.dma_start(out=m74_sb[:, :], in_=m74[:, :])
        eblk_sb = singles.tile([74, 3 * 128], f32)
        nc.sync.dma_start(out=eblk_sb[:, :], in_=eblk[:, :])
        idx_sb = singles.tile([128, Ftot], mybir.dt.int16)
        nc.sync.dma_start(out=idx_sb[:, :], in_=idx_in[:, :])
        nc.sync.dma_start(out=buf[:, 0:NCONST], in_=consts[:, :])

        foff = 0
        for p in range(1, NP + 1):
            P = int(PT[p])
            if p == 1:
                lrg = lr1_sb
            else:
                F = 2 * P // 16
                lrg = work.tile([128, 2 * PTmax], f32, tag="lrg")
                nc.gpsimd.ap_gather(
                    out_ap=lrg[:, : 2 * P],
                    in_ap=buf[:, :],
                    idxs_ap=idx_sb[:, foff : foff + F],
                    channels=128,
                    num_elems=S,
                    d=1,
                    num_idxs=2 * P,
                )
                foff += F
            ps_l = psum.tile([100, PTmax], f32, tag="ps_l")
            nc.tensor.matmul(ps_l[:, :P], wl_sb[:, :], lrg[:, 0:P],
                             start=True, stop=True)
            ps_r = psum.tile([100, PTmax], f32, tag="ps_r")
            nc.tensor.matmul(ps_r[:, :P], wr_sb[:, :], lrg[:, P : 2 * P],
                             start=True, stop=True)
            lcp = work.tile([100, PTmax], f32, tag="lcp")
            nc.vector.tensor_copy(lcp[:, :P], ps_l[:, :P])
            outer = work.tile([100, PTmax], f32, tag="outer")
            nc.vector.tensor_mul(outer[:, :P], lcp[:, :P], ps_r[:, :P])
            ps_z = psum.tile([74, PTmax], f32, tag="ps_z")
            nc.tensor.matmul(ps_z[:, :P], wmat_sb[:, :], outer[:, :P],
                             start=True, stop=True)
            if p == NP:
                zsb = work.tile([74, PTmax], f32, tag="zsb")
                nc.scalar.copy(zsb[:, :P], ps_z[:, :P])
                nc.sync.dma_start(out=outz[:, :], in_=zsb[:, :P])
                continue
            e = work.tile([74, PTmax], f32, tag="e")
            nc.scalar.activation(e[:, :P], ps_z[:, :P], mybir.ActivationFunctionType.Exp)
            ps_s = psum.tile([74, PTmax], f32, tag="ps_s")
            nc.tensor.matmul(ps_s[:, :P], m74_sb[:, :], e[:, :P],
                             start=True, stop=True)
            rz = work.tile([74, PTmax], f32, tag="rz")
            nc.vector.reciprocal(rz[:, :P], ps_s[:, :P])
            st = work.tile([74, PTmax], f32, tag="st")
            nc.vector.tensor_mul(st[:, :P], e[:, :P], rz[:, :P])
            b0 = int(_CUR_BASE[p])
            for o in range(NO):
                ps_b = psum.tile([128, PTmax], f32, tag=f"ps_b{o}")
                nc.tensor.matmul(ps_b[:, :P], eblk_sb[:, o * 128 : (o + 1) * 128],
                                 st[:, :P], start=True, stop=True)
                nc.scalar.copy(buf[:, b0 + o * P : b0 + (o + 1) * P], ps_b[:, :P])
    nc.finalize()
    return nc


_CUR_BASE = None


def kernel(op_table, cats, ops, lits, left, right, mask):
    global _CUR_BASE, LAST_RESULTS
    op_table = np.asarray(op_table, np.float32)
    plan = _plan(np.asarray(cats), np.asarray(ops), np.asarray(lits),
                 np.asarray(left), np.asarray(right), np.asarray(mask))
    PT, base, S, Ftot = plan["PT"], plan["base"], plan["S"], plan["Ftot"]
    _CUR_BASE = base

    nc = _build_nc(S, PT, Ftot)

    # Host-built constant operands.
    consts = np.zeros((128, NCONST), np.float32)
    for g in range(8):
        consts[16 * g : 16 * g + NI, :NI] = np.eye(NI, dtype=np.float32)
    repl = np.kron(np.eye(NI), np.ones((1, NI))).astype(np.float32)   # [10,100] l_a at 10a+b
    reprm = np.tile(np.eye(NI), (1, NI)).astype(np.float32)           # [10,100] r_b at 10a+b
    wl128 = np.zeros((128, 100), np.float32)
    wr128 = np.zeros((128, 100), np.float32)
    for g in range(8):
        wl128[16 * g : 16 * g + NI, :] = repl
        wr128[16 * g : 16 * g + NI, :] = reprm
    wmat = np.zeros((100, 74), np.float32)
    w30 = op_table.transpose(1, 2, 0, 3).reshape(100, 30)
    for o in range(NO):
        wmat[:, o * 32 : o * 32 + NI] = w30[:, o * NI : (o + 1) * NI]
    m74 = np.zeros((74, 74), np.float32)
    for o in range(NO):
        r0 = o * 32
        m74[r0 : r0 + NI, r0 : r0 + 32 if o < 2 else 74] = 1.0
    # columns (output rows) o*32+k for ALL k in the o block get the sum, so the
    # reciprocal never sees a zero from the unused rows 10..31.
    eblk = np.zeros((74, 3 * 128), np.float32)
    for o in range(NO):
        for g in range(8):
            for i in range(NI):
                eblk[o * 32 + i, o * 128 + 16 * g + i] = 1.0

    in_maps = []
    for c in range(NCORES):
        in_maps.append({
            "consts": consts, "wl": wl128, "wr": wr128, "wmat": wmat,
            "m74": m74, "eblk": eblk,
            "idx": np.ascontiguousarray(plan["idx"][c]),
            "lr1": np.ascontiguousarray(plan["lr1"][c]),
        })

    res = run_bass_kernel_spmd(nc, in_maps, list(range(NCORES)), trace=TRACE)
    LAST_RESULTS = res

    # Assemble the full (B, NI) output on the host (index selection only).
    out = np.zeros((B, NI), np.float32)
    litsc, is_lit = plan["litsc"], plan["is_lit"]
    lit_rows = np.nonzero(cats[:, 0] == 0)[0]
    lr_active = is_lit[lit_rows, 0]
    oh = 10.0 * np.eye(NI, dtype=np.float32)[litsc[lit_rows, 0]]
    out[lit_rows] = np.where(lr_active[:, None], oh, 0.0)

    r10, opsc = plan["r10"], plan["opsc"]
    core10, ul10 = plan["core10"], plan["ucol10"]
    for c in range(NCORES):
        z = np.asarray(res.results[c]["outz"])  # (74, PT10)
        selmask = core10 == c
        rows = r10[selmask]
        cc = ul10[selmask]
        o = opsc[rows, 0]
        zc = z[:, cc]
        sel = np.stack([zc[i * 32 : i * 32 + NI, :] for i in range(NO)])
        out[rows] = sel[o, :, np.arange(len(rows))]
    return out


# revision 10
# speedup vs baseline: 1.7316x; 1.2176x over previous
"""Trainium2 Bass kernel for nn_CruxMiniCircuit (gnn_message_passing).

Reference semantics: B independent rows; each row is a circuit of N nodes
(literal nodes hold a fixed one-hot distribution over 10 ints, op nodes
combine left/right child distributions through a per-op bilinear table
followed by softmax).  The reference runs 10 synchronous passes over all
nodes and returns only the root (node 0) logits per row.

Only node 0's dependency cone matters, so the host precomputes per-pass
integer worklists / gather indices; the device does all float math.

v2 design (latency-oriented; the v1 baseline spent most of the critical
path issuing 16 SBUF->SBUF DMAs per pass from the sync engine):

- The per-core value buffer buf[128, S] is REPLICATED across the eight
  16-partition GPSIMD blocks: the scatter at the end of each pass is a
  TensorE broadcast matmul (stationary E_o[74->128]) into PSUM followed
  by one ScalarE copy per op slice.  No DMAs inside the pass loop.
- The per-pass child gather is ONE ap_gather over all 128 channels with
  num_idxs = 2*PT: block g's channels carry real slot indices for the
  columns block g owns and ZSLOT (a zero vector) elsewhere.  A stationary
  Wl/Wr[128, 100] with the replicate pattern repeated per block then sums
  blockwise-disjoint contributions, yielding the left/right operand
  replication in one matmul each - the group-concat DMAs of v1 vanish.
- softmax: exp (ScalarE) -> block-sum matmul M74[74->74] -> reciprocal +
  multiply (VectorE) -> broadcast-scatter matmuls.
- Slot numbering is global per pass (no per-group padding): slot of
  update u with op o at pass p is base[p] + o*PT_p + u.

Sharding: pure data parallel over the batch dim (B=2048 -> 256 rows on
each of the 8 NeuronCores).  No collectives are needed.
"""

import sys
from contextlib import ExitStack

import numpy as np

sys.path.insert(0, "/opt/trn_rl_repo")

import concourse.bass as bass
import concourse.tile as tile
from concourse import bacc, mybir
from concourse.bass_utils import run_bass_kernel_spmd

B, N = 2048, 1023
NI, NO, NP = 10, 3, 10  # n_ints, n_ops, n_passes
NCORES = 8
RPC = B // NCORES  # rows per core
ZSLOT = NI  # value-buffer slot holding the zero vector
NCONST = NI + 1  # slots 0..9 = one-hot e_k, slot 10 = zeros

TRACE = False  # set True (e.g. from test.py) to profile the HW run
LAST_RESULTS = None  # BassKernelResults of the last run (exec_time_ns etc.)


def _plan(cats, ops, lits, left, right, mask):
    """Integer-only preprocessing: worklists, column assignment, gather idxs."""
    left = np.clip(left.astype(np.int64), 0, N - 1)
    right = np.clip(right.astype(np.int64), 0, N - 1)
    opsc = np.clip(ops.astype(np.int64), 0, NO - 1)
    litsc = np.clip(lits.astype(np.int64), 0, NI - 1)
    m = mask.astype(bool)
    is_lit = (cats == 0) & m
    is_opa = (cats == 1) & m
    const_slot = np.where(is_lit, litsc, ZSLOT)

    # Worklists W[p]: (row, node) updates needed at pass p.
    Wr = [None] * (NP + 1)
    Wn = [None] * (NP + 1)
    r10 = np.nonzero(cats[:, 0] == 1)[0].astype(np.int64)
    Wr[NP], Wn[NP] = r10, np.zeros(len(r10), np.int64)
    need = np.zeros((B, N), bool)
    for p in range(NP, 1, -1):
        r, n = Wr[p], Wn[p]
        cr = np.concatenate([r, r])
        cn = np.concatenate([left[r, n], right[r, n]])
        keep = is_opa[cr, cn]
        need[:] = False
        need[cr[keep], cn[keep]] = True
        rr, nn = np.nonzero(need)
        Wr[p - 1], Wn[p - 1] = rr.astype(np.int64), nn.astype(np.int64)

    # Per-core column ids (order of appearance) and per-pass padded width PT.
    PT = np.zeros(NP + 1, np.int64)
    ucol = [None] * (NP + 1)
    for p in range(1, NP + 1):
        r = Wr[p]
        core = r // RPC
        order = np.argsort(core, kind="stable")
        u = np.arange(len(r), dtype=np.int64)
        if len(r):
            cs = core[order]
            first = np.r_[True, cs[1:] != cs[:-1]]
            seg = np.nonzero(first)[0]
            uu = u - seg[np.cumsum(first) - 1]
            ul = np.empty(len(r), np.int64)
            ul[order] = uu
        else:
            ul = u
        cnt = np.bincount(core, minlength=NCORES) if len(r) else np.zeros(NCORES, np.int64)
        mx = int(cnt.max()) if len(r) else 0
        PT[p] = max(16, -(-mx // 16) * 16)  # mult of 16 -> idx words fill F exactly
        ucol[p] = ul

    # Slot bases: pass p stores 3*PT[p] slots (p = 1..NP-1).
    base = np.zeros(NP + 1, np.int64)
    base[1] = NCONST
    for p in range(2, NP + 1):
        base[p] = base[p - 1] + 3 * PT[p - 1]
    S = int(base[NP - 1] + 3 * PT[NP - 1])
    assert S <= 32000, f"value buffer too large for gather: {S}"

    idx_parts = []
    Fs = []
    slot_prev = np.full((B, N), -1, np.int64)
    lr1 = None
    for p in range(1, NP + 1):
        r, n, ul = Wr[p], Wn[p], ucol[p]
        core = r // RPC
        P = int(PT[p])
        Q8 = P // 8  # columns per 16-partition block (may not be mult of 8; fine)
        g = np.minimum(ul // max(Q8, 1), 7)
        lch, rch = left[r, n], right[r, n]
        if p == 1:
            lidx = const_slot[r, lch]
            ridx = const_slot[r, rch]
        else:
            lidx = np.where(is_opa[r, lch],
                            base[p - 1] + opsc[r, lch] * PT[p - 1] + slot_prev[r, lch],
                            const_slot[r, lch])
            ridx = np.where(is_opa[r, rch],
                            base[p - 1] + opsc[r, rch] * PT[p - 1] + slot_prev[r, rch],
                            const_slot[r, rch])
        if p == 1:
            # pass-1 inputs are compile-time one-hots; built host-side, shipped
            # replicated into the block layout: lr1[c, 16g+i, u] for block g.
            eyeext = np.concatenate([np.eye(NI, dtype=np.float32),
                                     np.zeros((NI, 1), np.float32)], axis=1)
            arr = np.full((NCORES, 2 * P), ZSLOT, np.int64)
            arr[core, ul] = lidx
            arr[core, P + ul] = ridx
            gcol = np.minimum(np.concatenate([np.arange(P), np.arange(P)]) // max(Q8, 1), 7)
            oh = eyeext[:, arr]  # (10, NCORES, 2P)
            big = np.zeros((NCORES, 8, 16, 2 * P), np.float32)
            for gg in range(8):
                sel = gcol == gg
                big[:, gg, :NI, sel] = oh[:, :, sel].transpose(2, 1, 0)
            lr1 = big.reshape(NCORES, 128, 2 * P)
        else:
            # Gather idx array: block g carries real slots for its own column
            # span, ZSLOT elsewhere.  Wrapped: idx j -> [16g + j%16, j//16].
            F = 2 * P // 16
            arr = np.full((NCORES, 8, 2 * P), ZSLOT, np.int64)
            arr[core, g, ul] = lidx
            arr[core, g, P + ul] = ridx
            w = arr.reshape(NCORES, 8, F, 16).transpose(0, 1, 3, 2).reshape(NCORES, 128, F)
            idx_parts.append(w.astype(np.int16))
            Fs.append(F)
        if p < NP:
            slot_prev = np.full((B, N), -1, np.int64)
            slot_prev[r, n] = ul

    idx_full = np.concatenate(idx_parts, axis=2)  # (NCORES, 128, sum F)

    return dict(
        PT=PT, base=base, S=S, idx=idx_full, Ftot=int(sum(Fs)), lr1=lr1,
        r10=r10, core10=Wr[NP] // RPC, ucol10=ucol[NP],
        opsc=opsc, litsc=litsc, is_lit=is_lit,
    )


def _build_nc(S, PT, Ftot):
    f32 = mybir.dt.float32
    f16 = mybir.dt.float16
    nc = bacc.Bacc(None)
    PTmax = int(max(PT[1:]))
    P1 = int(PT[1])
    P10 = int(PT[NP])

    consts = nc.dram_tensor("consts", [128, NCONST], f32, kind="ExternalInput")
    wl = nc.dram_tensor("wl", [128, 100], f16, kind="ExternalInput")
    wr = nc.dram_tensor("wr", [128, 100], f16, kind="ExternalInput")
    wmat = nc.dram_tensor("wmat", [100, 74], f16, kind="ExternalInput")
    m74 = nc.dram_tensor("m74", [74, 74], f16, kind="ExternalInput")
    eblk = nc.dram_tensor("eblk", [74, 3 * 128], f16, kind="ExternalInput")
    idx_in = nc.dram_tensor("idx", [128, Ftot], mybir.dt.int16, kind="ExternalInput")
    lr1_in = nc.dram_tensor("lr1", [128, 2 * P1], f16, kind="ExternalInput")
    outz = nc.dram_tensor("outz", [74, P10], f32, kind="ExternalOutput")

    with ExitStack() as ctx:
        tc = ctx.enter_context(tile.TileContext(nc))
        singles = ctx.enter_context(tc.tile_pool(name="singles", bufs=1))
        work = ctx.enter_context(tc.tile_pool(name="work", bufs=1))
        psum = ctx.enter_context(tc.tile_pool(name="psum", bufs=1, space="PSUM"))

        buf = singles.tile([128, S], f32)
        # No full memset: the gather only ever touches the consts region and
        # slots written by the previous pass's scatter.
        wl_sb = singles.tile([128, 100], f16)
        nc.sync.dma_start(out=wl_sb[:, :], in_=wl[:, :])
        wr_sb = singles.tile([128, 100], f16)
        nc.sync.dma_start(out=wr_sb[:, :], in_=wr[:, :])
        lr1_sb = singles.tile([128, 2 * P1], f16)
        nc.sync.dma_start(out=lr1_sb[:, :], in_=lr1_in[:, :])
        wmat_sb = singles.tile([100, 74], f16)
        nc.sync.dma_start(out=wmat_sb[:, :], in_=wmat[:, :])
        m74_sb = singles.tile([74, 74], f16)
        nc.sync.dma_start(out=m74_sb[:, :], in_=m74[:, :])
        eblk_sb = singles.tile([74, 3 * 128], f16)
        nc.sync.dma_start(out=eblk_sb[:, :], in_=eblk[:, :])
        idx_sb = singles.tile([128, Ftot], mybir.dt.int16)
        nc.sync.dma_start(out=idx_sb[:, :], in_=idx_in[:, :])
        nc.sync.dma_start(out=buf[:, 0:NCONST], in_=consts[:, :])

        foff = 0
        for p in range(1, NP + 1):
            P = int(PT[p])
            if p == 1:
                lrg16 = lr1_sb
            else:
                F = 2 * P // 16
                lrg = work.tile([128, 2 * PTmax], f32, tag="lrg")
                nc.gpsimd.ap_gather(
                    out_ap=lrg[:, : 2 * P],
                    in_ap=buf[:, :],
                    idxs_ap=idx_sb[:, foff : foff + F],
                    channels=128,
                    num_elems=S,
                    d=1,
                    num_idxs=2 * P,
                )
                foff += F
                lrg16 = work.tile([128, 2 * PTmax], f16, tag="lrg16")
                nc.vector.tensor_copy(lrg16[:, : 2 * P], lrg[:, : 2 * P])
            ps_l = psum.tile([100, PTmax], f32, tag="ps_l")
            nc.tensor.matmul(ps_l[:, :P], wl_sb[:, :], lrg16[:, 0:P],
                             start=True, stop=True)
            ps_r = psum.tile([100, PTmax], f32, tag="ps_r")
            nc.tensor.matmul(ps_r[:, :P], wr_sb[:, :], lrg16[:, P : 2 * P],
                             start=True, stop=True)
            lcp = work.tile([100, PTmax], f16, tag="lcp")
            nc.vector.tensor_copy(lcp[:, :P], ps_l[:, :P])
            outer = work.tile([100, PTmax], f16, tag="outer")
            nc.vector.tensor_mul(outer[:, :P], lcp[:, :P], ps_r[:, :P])
            ps_z = psum.tile([74, PTmax], f32, tag="ps_z")
            nc.tensor.matmul(ps_z[:, :P], wmat_sb[:, :], outer[:, :P],
                             start=True, stop=True)
            if p == NP:
                zsb = work.tile([74, PTmax], f32, tag="zsb")
                nc.scalar.copy(zsb[:, :P], ps_z[:, :P])
                nc.sync.dma_start(out=outz[:, :], in_=zsb[:, :P])
                continue
            e = work.tile([74, PTmax], f16, tag="e")
            nc.scalar.activation(e[:, :P], ps_z[:, :P], mybir.ActivationFunctionType.Exp)
            ps_s = psum.tile([74, PTmax], f32, tag="ps_s")
            nc.tensor.matmul(ps_s[:, :P], m74_sb[:, :], e[:, :P],
                             start=True, stop=True)
            rz = work.tile([74, PTmax], f32, tag="rz")
            nc.vector.reciprocal_approx_fast(rz[:, :P], ps_s[:, :P])
            st = work.tile([74, PTmax], f16, tag="st")
            nc.vector.tensor_mul(st[:, :P], e[:, :P], rz[:, :P])
            b0 = int(_CUR_BASE[p])
            # scatter: broadcast matmuls into two packed PSUM banks, then two
            # ScalarE copies into the value buffer (no DMAs in the pass loop).
            ps_b01 = psum.tile([128, 2 * PTmax], f32, tag="ps_b01")
            for o in range(2):
                nc.tensor.matmul(ps_b01[:, o * P : (o + 1) * P],
                                 eblk_sb[:, o * 128 : (o + 1) * 128],
                                 st[:, :P], start=True, stop=True)
            ps_b2 = psum.tile([128, PTmax], f32, tag="ps_b2")
            nc.tensor.matmul(ps_b2[:, :P], eblk_sb[:, 2 * 128 : 3 * 128],
                             st[:, :P], start=True, stop=True)
            nc.scalar.copy(buf[:, b0 : b0 + 2 * P], ps_b01[:, : 2 * P])
            nc.scalar.copy(buf[:, b0 + 2 * P : b0 + 3 * P], ps_b2[:, :P])
    nc.finalize()
    return nc


_CUR_BASE = None


def kernel(op_table, cats, ops, lits, left, right, mask):
    global _CUR_BASE, LAST_RESULTS
    op_table = np.asarray(op_table, np.float32)
    plan = _plan(np.asarray(cats), np.asarray(ops), np.asarray(lits),
                 np.asarray(left), np.asarray(right), np.asarray(mask))
    PT, base, S, Ftot = plan["PT"], plan["base"], plan["S"], plan["Ftot"]
    _CUR_BASE = base

    nc = _build_nc(S, PT, Ftot)

    # Host-built constant operands.
    consts = np.zeros((128, NCONST), np.float32)
    for g in range(8):
        consts[16 * g : 16 * g + NI, :NI] = np.eye(NI, dtype=np.float32)
    repl = np.kron(np.eye(NI), np.ones((1, NI))).astype(np.float32)   # [10,100] l_a at 10a+b
    reprm = np.tile(np.eye(NI), (1, NI)).astype(np.float32)           # [10,100] r_b at 10a+b
    wl128 = np.zeros((128, 100), np.float32)
    wr128 = np.zeros((128, 100), np.float32)
    for g in range(8):
        wl128[16 * g : 16 * g + NI, :] = repl
        wr128[16 * g : 16 * g + NI, :] = reprm
    wmat = np.zeros((100, 74), np.float32)
    w30 = op_table.transpose(1, 2, 0, 3).reshape(100, 30)
    for o in range(NO):
        wmat[:, o * 32 : o * 32 + NI] = w30[:, o * NI : (o + 1) * NI]
    m74 = np.zeros((74, 74), np.float32)
    for o in range(NO):
        r0 = o * 32
        m74[r0 : r0 + NI, r0 : r0 + 32 if o < 2 else 74] = 1.0
    # columns (output rows) o*32+k for ALL k in the o block get the sum, so the
    # reciprocal never sees a zero from the unused rows 10..31.
    eblk = np.zeros((74, 3 * 128), np.float32)
    for o in range(NO):
        for g in range(8):
            for i in range(NI):
                eblk[o * 32 + i, o * 128 + 16 * g + i] = 1.0

    in_maps = []
    for c in range(NCORES):
        in_maps.append({
            "consts": consts,
            "wl": wl128.astype(np.float16), "wr": wr128.astype(np.float16),
            "wmat": wmat.astype(np.float16), "m74": m74.astype(np.float16),
            "eblk": eblk.astype(np.float16),
            "idx": np.ascontiguousarray(plan["idx"][c]),
            "lr1": np.ascontiguousarray(plan["lr1"][c]).astype(np.float16),
        })

    res = run_bass_kernel_spmd(nc, in_maps, list(range(NCORES)), trace=TRACE)
    LAST_RESULTS = res

    # Assemble the full (B, NI) output on the host (index selection only).
    out = np.zeros((B, NI), np.float32)
    litsc, is_lit = plan["litsc"], plan["is_lit"]
    lit_rows = np.nonzero(cats[:, 0] == 0)[0]
    lr_active = is_lit[lit_rows, 0]
    oh = 10.0 * np.eye(NI, dtype=np.float32)[litsc[lit_rows, 0]]
    out[lit_rows] = np.where(lr_active[:, None], oh, 0.0)

    r10, opsc = plan["r10"], plan["opsc"]
    core10, ul10 = plan["core10"], plan["ucol10"]
    for c in range(NCORES):
        z = np.asarray(res.results[c]["outz"])  # (74, PT10)
        selmask = core10 == c
        rows = r10[selmask]
        cc = ul10[selmask]
        o = opsc[rows, 0]
        zc = z[:, cc]
        sel = np.stack([zc[i * 32 : i * 32 + NI, :] for i in range(NO)])
        out[rows] = sel[o, :, np.arange(len(rows))]
    return out
